# revision 27
# baseline (speedup 1.0000x reference)
"""Trainium2 Bass kernel for the MetricLearning pairwise loss.

Reference math:
    d2[i,j] = max(||x_i||^2 + ||x_j||^2 - 2 x_i.x_j, EPS)
    a = d2/(2k)/sigma^2 ; b = d2/(2k)/omega^2 ; c1 = k/2-1
    per_pair = same ? (-c1*log(a) + a/2) : (c1*log(b) - b/2)
    loss = sum_{i<j} per_pair

Split per pair (L = log d2):
    diff formula on every pair:  c1*(L + lnB) - (B/2)*d2
    same-pair correction:        -2c1*L - c1*(lnA+lnB) + ((A+B)/2)*d2
All terms linear in d2 and the pair counts are computed on the HOST in
fp64 over the exact quantized data; only the log sums need the device:
    S1 = sum L over cross-block pairs (one orientation each)
    S2 = sum L over full 256x256 diagonal blocks (diag pinned to EPS_D2)
    S3 = sum same-mask * L over full diagonal blocks (bf16 L)
    S4 = sum same-mask * L over block-boundary corners (i<j rows x cols)
    loss = c1*S1 + (c1/2)*(S2 - N*lnE) - c1*(S3 - N*lnE_bf) - 2c1*S4 + host
Triangle masks are gone: within-block sums use the double-count identity
sum_{i<j} = (sum_full - sum_diag)/2 (mask is symmetric), and the diagonal
is clamped so that d2_ii == EPS_D2 exactly (fp32-exact subtraction).

Rows are globally SORTED BY LABEL (runs < 128 rows), so same-label pairs
live within a block or in the 128-wide corner between consecutive blocks.

Per core (SPMD, K16 edge orientation): lhs blocks l0=2d, l1=2d+1; 9 tiles
per 128-row unit u: A=[l0 diag|edge l0-l1], F=[l1 diag|edge l1-l1+1],
plus 7 pure-cross tiles. The aug matmul (adds -sq_j/2, K=2 bf16 hi/lo)
issues FIRST in each PSUM group so every LDWEIGHTS hides under the
previous matmul's stream. Bulk input arrives as 5 grouped DMAs with 1KB
descriptors on two HW queues; dummy fp8 matmuls + a dummy Ln warm the PE
clock (HAM) and the ACT table during the fill.
"""

import numpy as np
import ml_dtypes

N = 4096
D = 1024
P = 128
NB = 16          # row blocks
BLK = 256        # rows per block
KC = D // P      # k chunks (8)
NCORES = 8

SIGMA = 0.2
OMEGA = 1.0
K_F = float(N)
C1 = K_F / 2.0 - 1.0                      # 2047
A_C = 1.0 / (2.0 * K_F * SIGMA * SIGMA)
B_C = 1.0 / (2.0 * K_F * OMEGA * OMEGA)
LOG_A = float(np.log(A_C))
LOG_B = float(np.log(B_C))
EPS_D2 = 256.0   # diagonal pin value; real off-diag d2 >= ~1500
LNE = float(np.log(EPS_D2))
LNE_BF = float(ml_dtypes.bfloat16(np.log(EPS_D2)))

# tiles per unit: (lhs_ls in {0,1}, first col slot, n slots, kind)
TILES = [
    (0, 0, 2, "A"),   # l0 diag + edge (l0,l1)
    (1, 1, 2, "F"),   # l1 diag + edge (l1,corner)
    (0, 3, 2, "X"),
    (0, 5, 2, "X"),
    (0, 7, 2, "X"),
    (0, 9, 1, "X"),
    (1, 10, 2, "X"),
    (1, 12, 2, "X"),
    (1, 14, 2, "X"),
]
# emission order: tile-major, u inner (matches DMA group arrival)
PHASES = [(ti, u) for ti in range(len(TILES)) for u in (0, 1)]
NPH = len(PHASES)  # 18

# acc column map
COL_X = {}     # phase -> cross-L col (coeff c1)
COL_D = {}     # diag-L col (coeff c1/2)
COL_M = {}     # diag mask-L col (coeff -c1)
COL_C = {}     # corner mask-L col (coeff -2c1)
_c = 0
for _pi, (_ti, _u) in enumerate(PHASES):
    COL_X[_pi] = _c; _c += 1
for _pi, (_ti, _u) in enumerate(PHASES):
    if TILES[_ti][3] != "X":
        COL_D[_pi] = _c; _c += 1
for _pi, (_ti, _u) in enumerate(PHASES):
    if TILES[_ti][3] != "X":
        COL_M[_pi] = _c; _c += 1
for _pi, (_ti, _u) in enumerate(PHASES):
    if TILES[_ti][3] != "X" and _u == 1:
        COL_C[_pi] = _c; _c += 1
ACC_W = 32
assert _c <= ACC_W

# bulk DMA slab groups (slot ranges) aligned to tile column sets, all on
# the sync HW queue in phase order. The scalar queue carries only the
# three small inputs: bulk issues there would block the ACT engine
# (DMA-issue instructions occupy the issuing engine, and semaphore-lane
# reuse can make them wait on in-flight transfers).
GROUPS = [(0, 2), (2, 1), (3, 2), (5, 2), (7, 2), (9, 3), (12, 2), (14, 2)]
GROUP_Q = ["sync"] * 8


def _partners(d):
    """Block orientation: edge {i,j} (i<j) owned by i if i+j odd else j."""
    l0, l1 = 2 * d, 2 * d + 1
    p8 = [j for j in range(l0 + 1, NB) if j % 2 == 1] + \
         [i for i in range(0, l0) if i % 2 == 0]
    p7 = [j for j in range(l1 + 1, NB) if j % 2 == 0] + \
         [i for i in range(0, l1) if i % 2 == 1]
    assert len(p8) == 8 and len(p7) == 7 and l1 in p8
    return l0, l1, p8, p7


def _core_slabs(d):
    """Slot -> block id. slot0=l0, slot1=l1, slot2=corner partner
    (l1+1 when it exists, so the consecutive-pair corner sits in tile F)."""
    l0, l1, p8, p7 = _partners(d)
    nxt = l1 + 1
    corner = nxt if nxt in p7 else p7[0]
    rest8 = [p for p in p8 if p != l1]
    rest7 = [p for p in p7 if p != corner]
    slabs = [l0, l1, corner] + rest8 + rest7
    assert len(slabs) == NB and len(set(slabs)) == NB
    return slabs


_PROG_CACHE = {}


def _build_program():
    if "nc" in _PROG_CACHE:
        return _PROG_CACHE["nc"]
    import concourse.bass as bass  # noqa: F401
    import concourse.bacc as bacc
    import concourse.mybir as mybir
    import concourse.tile as tile

    F32 = mybir.dt.float32
    BF16 = mybir.dt.bfloat16
    FP8 = mybir.dt.float8e4
    AF = mybir.ActivationFunctionType
    ALU = mybir.AluOpType
    DR = mybir.MatmulPerfMode.DoubleRow

    nc = bacc.Bacc("TRN2", target_bir_lowering=False, debug=False,
                   num_devices=NCORES)
    xg_d = [nc.dram_tensor(f"xg{i}", [P, KC, gs, BLK], FP8,
                           kind="ExternalInput").ap()
            for i, (_, gs) in enumerate(GROUPS)]
    aug_d = nc.dram_tensor("aug", [2, N], BF16, kind="ExternalInput").ap()
    lab_d = nc.dram_tensor("lab", [P, 640], BF16, kind="ExternalInput").ap()
    rowd_d = nc.dram_tensor("rowd", [P, 4 * 3], F32, kind="ExternalInput").ap()
    out_d = nc.dram_tensor("out", [P, ACC_W], F32, kind="ExternalOutput").ap()

    with tile.TileContext(nc) as tc:
        with (
            tc.tile_pool(name="persist", bufs=1) as persist,
            tc.tile_pool(name="lbuf", bufs=3) as lpool,
            tc.tile_pool(name="dscratch", bufs=2) as dscratch,
            tc.tile_pool(name="psum", bufs=5, space="PSUM") as psum,
            tc.tile_pool(name="psumw", bufs=1, space="PSUM") as psumw,
        ):
            xall = persist.tile([P, KC, NB, BLK], FP8, tag="xall")
            labb = persist.tile([P, 640], BF16, tag="labb")
            augs = persist.tile([2, N], BF16, tag="augs")
            rd = persist.tile([P, 4 * 3], F32, tag="rd")
            ones2 = persist.tile([2, P], BF16, tag="ones2")
            junk = persist.tile([P, 2, 512], FP8, tag="junk")
            acc = persist.tile([P, ACC_W], F32, tag="acc")
            ldump = persist.tile([P, 512], BF16, tag="ldump")

            # ones2 first (tiny) so PE warm-up can start almost immediately
            nc.gpsimd.memset(ones2[:], 1.0)
            nc.gpsimd.memset(junk[:], 0.0)
            nc.gpsimd.memset(acc[:], 0.0)

            # small inputs first on the scalar queue (aug gates every phase)
            nc.scalar.dma_start(out=augs[:], in_=aug_d[:])
            nc.scalar.dma_start(out=rd[:], in_=rowd_d[:])
            nc.scalar.dma_start(out=labb[:], in_=lab_d[:])

            # bulk slab groups on the sync queue, phase order
            for gi, (s0, gs) in enumerate(GROUPS):
                nc.sync.dma_start(out=xall[:, :, s0:s0 + gs, :], in_=xg_d[gi])

            # PE warm-up (HAM) on junk zeros while DMAs fill SBUF. Sized so
            # the phases start with ~1MB of supply already buffered: a PE
            # gap here re-throttles the clock for ~3.4us windows.
            wt = psumw.tile([P, 512], F32, tag="warm")
            for _ in range(5):
                # tiny bf16 warm-ups gated only on the 100ns ones2 memset:
                # they bridge the PE until the junk memset completes
                nc.tensor.matmul(wt[:, 0:128], ones2[:, :], ones2[:, :],
                                 start=True, stop=True)
            for _ in range(11):
                nc.tensor.matmul(wt[:], junk[:, :, 0:128], junk[:, :, 0:512],
                                 start=True, stop=True, perf_mode=DR)
            # dummy Ln preloads the ACT table early; same scale sign and
            # operand shapes as the real calls so no second load later
            nc.scalar.activation(ldump[:, 0:3], acc[:, 28:31], AF.Ln,
                                 bias=rd[:, 0:1], scale=-2.0,
                                 accum_out=acc[:, ACC_W - 1:ACC_W])

            for pi, (ti, u) in enumerate(PHASES):
                ls, slot0, ns, kind = TILES[ti]
                wid = ns * BLK
                clo = slot0 * BLK
                g = 2 * ls + u
                sq_ap = rd[:, 3 * g + 0:3 * g + 1]
                lb_ap = rd[:, 3 * g + 1:3 * g + 2]
                th_ap = rd[:, 3 * g + 2:3 * g + 3]

                t = psum.tile([P, wid], F32, tag="gram")
                # aug first: every LDWEIGHTS hides under a matmul stream
                nc.tensor.matmul(t[:], ones2[:, :], augs[:, clo:clo + wid],
                                 start=True, stop=False)
                for kc2 in range(KC // 2):
                    nc.tensor.matmul(
                        t[:],
                        xall[:, 2 * kc2:2 * kc2 + 2, ls, 128 * u:128 * (u + 1)],
                        xall[:, 2 * kc2:2 * kc2 + 2, slot0:slot0 + ns, :],
                        start=False, stop=(kc2 == KC // 2 - 1),
                        perf_mode=DR,
                    )

                if kind == "X":
                    ld = lpool.tile([P, 512], BF16, tag="L")
                    if pi >= NPH - 2:
                        # final phases: ACT accum keeps the tail short
                        nc.scalar.activation(ld[:, 0:wid], t[:], AF.Ln,
                                             bias=sq_ap, scale=-2.0,
                                             accum_out=acc[:, COL_X[pi]:
                                                           COL_X[pi] + 1])
                    else:
                        # L-sum via DVE reduce: keeps ACC_READs off ACT
                        nc.scalar.activation(ld[:, 0:wid], t[:], AF.Ln,
                                             bias=sq_ap, scale=-2.0)
                        nc.vector.tensor_reduce(
                            acc[:, COL_X[pi]:COL_X[pi] + 1], ld[:, 0:wid],
                            axis=mybir.AxisListType.X, op=ALU.add)
                else:
                    # diag half: pin the diagonal to d2 == EPS_D2 via min
                    t2 = dscratch.tile([P, 256], F32, tag="t2")
                    nc.vector.tensor_scalar(t2[:], t[:, 0:256], th_ap, None,
                                            ALU.min)
                    lb = lpool.tile([P, 512], BF16, tag="L")
                    nc.scalar.activation(lb[:, 0:256], t2[:], AF.Ln,
                                         bias=sq_ap, scale=-2.0,
                                         accum_out=acc[:, COL_D[pi]:
                                                       COL_D[pi] + 1])
                    nc.scalar.activation(lb[:, 256:512], t[:, 256:512], AF.Ln,
                                         bias=sq_ap, scale=-2.0,
                                         accum_out=acc[:, COL_X[pi]:
                                                       COL_X[pi] + 1])
                    # same-label mask over diag (and corner when u=1)
                    mw = 384 if u == 1 else 256
                    lwin = labb[:, clo:clo + mw]
                    m = dscratch.tile([P, 384], BF16, tag="m")
                    nc.vector.tensor_scalar(m[:, 0:mw], lwin, lb_ap, None,
                                            ALU.is_equal)
                    prod = dscratch.tile([P, 384], BF16, tag="prod")
                    nc.vector.tensor_tensor(prod[:, 0:mw], m[:, 0:mw],
                                            lb[:, 0:mw], ALU.mult)
                    nc.vector.tensor_reduce(
                        acc[:, COL_M[pi]:COL_M[pi] + 1], prod[:, 0:256],
                        axis=mybir.AxisListType.X, op=ALU.add)
                    if u == 1:
                        nc.vector.tensor_reduce(
                            acc[:, COL_C[pi]:COL_C[pi] + 1],
                            prod[:, 256:384],
                            axis=mybir.AxisListType.X, op=ALU.add)

                if pi == NPH - 3:
                    # ship the early-complete acc columns while the last
                    # phases run; only cols 16:32 remain for the tail DMA
                    nc.sync.dma_start(out=out_d[:, 0:16], in_=acc[:, 0:16])

            nc.sync.dma_start(out=out_d[:, 16:ACC_W], in_=acc[:, 16:ACC_W])

    nc.compile()
    _PROG_CACHE["nc"] = nc
    return nc


def _host_prep(outputs, labels):
    """Sort rows by label, build per-core inputs + the host fp64 terms."""
    x = np.asarray(outputs, dtype=np.float32)
    lab = np.asarray(labels)
    assert x.shape == (N, D)
    perm = np.argsort(lab, kind="stable")
    xp = x[perm]
    labp = lab[perm].astype(np.float64)

    _, starts, counts = np.unique(labp, return_index=True, return_counts=True)
    assert counts.max() <= 128, f"label run {counts.max()} exceeds corner"

    xq8 = xp.astype(ml_dtypes.float8_e4m3)
    xq = xq8.astype(np.float64)
    # True (unquantized) norms make d2 = sq_i + sq_j - 2*xq_i.xq_j unbiased
    sq = (xp.astype(np.float64) ** 2).sum(axis=1)

    # host analytic terms (exact over quantized gram + exact norms)
    P_total = N * (N - 1) // 2
    P_same = float((counts * (counts - 1) // 2).sum())
    Sg_all = (np.dot(xq.sum(0), xq.sum(0)) - (xq * xq).sum()) / 2.0
    Sd2_all = (N - 1) * sq.sum() - 2.0 * Sg_all
    vs = np.add.reduceat(xq, starts, axis=0)
    qs_run = np.add.reduceat((xq * xq).sum(1), starts)
    sq_run = np.add.reduceat(sq, starts)
    Sg_same = ((vs * vs).sum(1) - qs_run).sum() / 2.0
    Sd2_same = ((counts - 1) * sq_run).sum() - 2.0 * Sg_same
    host_add = (C1 * LOG_B * P_total
                - C1 * (LOG_A + LOG_B) * P_same
                - 0.5 * B_C * Sd2_all
                + 0.5 * (A_C + B_C) * Sd2_same
                - 0.5 * C1 * N * LNE
                + C1 * N * LNE_BF)

    xt_q = np.ascontiguousarray(xq8.T)                             # [D, N]
    neg_half = -0.5 * sq
    hi = neg_half.astype(ml_dtypes.bfloat16)
    lo = (neg_half - hi.astype(np.float64)).astype(ml_dtypes.bfloat16)

    in_maps = []
    for d in range(NCORES):
        slabs = _core_slabs(d)
        cols = np.concatenate(
            [np.arange(b * BLK, (b + 1) * BLK) for b in slabs])
        # [P, KC, NB, BLK]: feature f = kc*128 + p
        xtp = np.ascontiguousarray(
            xt_q[:, cols].reshape(KC, P, NB, BLK).transpose(1, 0, 2, 3))
        aug = np.stack([hi[cols], lo[cols]])                       # [2, N]
        # label row for slot0(256) | slot1(256) | slot2 first 128,
        # pre-broadcast across partitions so no device-side bcast is needed
        lcols = np.concatenate([cols[0:512], cols[2 * BLK:2 * BLK + 128]])
        labrow = np.broadcast_to(
            labp[lcols].astype(ml_dtypes.bfloat16)[None, :], (P, 640))

        rowd = np.zeros((P, 4 * 3), dtype=np.float64)
        for g, (slab, u) in enumerate(((0, 0), (0, 1), (1, 0), (1, 1))):
            rows = slabs[slab] * BLK + 128 * u + np.arange(P)
            sqr = sq[rows]
            rowd[:, 3 * g + 0] = sqr
            rowd[:, 3 * g + 1] = labp[rows]
            rowd[:, 3 * g + 2] = (sqr - EPS_D2) / 2.0
        im = {
            "aug": np.ascontiguousarray(aug),
            "lab": np.ascontiguousarray(labrow),
            "rowd": rowd.astype(np.float32),
        }
        for gi, (s0, gs) in enumerate(GROUPS):
            im[f"xg{gi}"] = np.ascontiguousarray(xtp[:, :, s0:s0 + gs, :])
        in_maps.append(im)
    return in_maps, host_add


def _combine(results, host_add):
    ncx = len(COL_X)
    s = np.zeros(4, dtype=np.float64)
    for r in results:
        o = np.asarray(r["out"], dtype=np.float64)
        s[0] += o[:, 0:ncx].sum()
        s[1] += o[:, list(COL_D.values())].sum()
        s[2] += o[:, list(COL_M.values())].sum()
        s[3] += o[:, list(COL_C.values())].sum()
    total = (C1 * s[0] + 0.5 * C1 * s[1] - C1 * s[2] - 2.0 * C1 * s[3]
             + host_add)
    return np.asarray(total, dtype=np.float32)


def kernel(**inputs):
    from concourse.bass_utils import run_bass_kernel_spmd
    nc = _build_program()
    in_maps, host_add = _host_prep(inputs["outputs"], inputs["labels"])
    res = run_bass_kernel_spmd(nc, in_maps, core_ids=list(range(NCORES)))
    return _combine(res.results, host_add)
